# revision 1
# baseline (speedup 1.0000x reference)
"""TRN2 Bass kernel: cross-attention (nn_CrossAttention_42047729828228).

Computes, per batch b:
  q = x @ Wq.T ; k = key @ Wk.T ; v = value @ Wv.T      (heads H=8, C=64)
  sim = einsum('nhc,mhc->hnm', q, k) * SCALE
  sim = where(mask, sim, -inf) + L1*box + L2*road
  out = einsum('hnm,mhc->nhc', softmax(sim, -1), v) @ Wo.T + bo

Device strategy: data-parallel over batch B=32 across 8 NeuronCores (4 each).
Per core, one Tile program processes its 4 batches.

Key algebraic simplifications (exact):
 - road bias is constant along the softmax (key) axis -> cancels; dropped.
 - SCALE folded into Wq host-side.
 - key mask folded into an additive per-key bias (0 / -1e9), applied with the
   box bias in one fused DVE op; exp(-1e9) == 0 exactly in fp32.

On-chip layout: scores are built transposed, simT (m on partitions, n free),
so the sim matmul streams n (free dim 512 -> full-rate fp32r) and the AV
matmul consumes exp(simT) directly as its moving operand. The softmax
denominator is accumulated into one (8, n) psum via one-hot lhsT matmuls,
reciprocal'd on DVE, broadcast back to 128 partitions with a constant
pair-selector matmul, and applied as the PSUM->SBUF move of the AV output.
"""

import os
import sys

import numpy as np

sys.path.insert(0, "/opt/trn_rl_repo")

import concourse.bass as bass  # noqa: E402
import concourse.bacc as bacc  # noqa: E402
import concourse.mybir as mybir  # noqa: E402
import concourse.tile as tile  # noqa: E402

F32 = mybir.dt.float32
F32R = mybir.dt.float32r
AF = mybir.ActivationFunctionType
ALU = mybir.AluOpType

# Problem shapes (hardcoded; see module docstring).
B, N, M = 32, 1536, 80
QD, KD, VD = 320, 768, 768
H, C = 8, 64
INNER = H * C  # 512
OD = QD  # 320
SCALE = C**-0.5
NCORES = 8
BP = B // NCORES  # 4 batches per core
NCH = 512  # n-chunk (matmul moving dim)
NT = 128  # n-tile
NCHUNKS = N // NCH  # 3
NTT = NCH // NT  # 4
NPAIR = H // 2  # 4 head pairs
IC = INNER // 128  # 4 i-chunks
KC = KD // 128  # 6 kd-chunks
MASK_NEG = -1.0e9


def build_program(split_waits=True):  # split_waits kept for API compat; Bacc.compile() handles it
    nc = bacc.Bacc("TRN2", target_bir_lowering=False, debug=False, num_devices=NCORES)

    x_d = nc.dram_tensor("x", [BP, N, QD], F32, kind="ExternalInput").ap()
    key_d = nc.dram_tensor("key", [BP, M, KD], F32, kind="ExternalInput").ap()
    val_d = nc.dram_tensor("value", [BP, M, VD], F32, kind="ExternalInput").ap()
    mb_d = nc.dram_tensor("maskbias", [BP, M], F32, kind="ExternalInput").ap()
    box_d = nc.dram_tensor("box", [BP, N, M], F32, kind="ExternalInput").ap()
    wqt_d = nc.dram_tensor("WqT", [QD, INNER], F32R, kind="ExternalInput").ap()
    wkt_d = nc.dram_tensor("WkT", [KD, INNER], F32R, kind="ExternalInput").ap()
    wvt_d = nc.dram_tensor("WvT", [VD, INNER], F32R, kind="ExternalInput").ap()
    wot_d = nc.dram_tensor("WoT", [INNER, OD], F32R, kind="ExternalInput").ap()
    bo_d = nc.dram_tensor("bo", [OD], F32, kind="ExternalInput").ap()
    ident_d = nc.dram_tensor("ident", [128, 128], F32, kind="ExternalInput").ap()
    zoneh_d = nc.dram_tensor("zoneh", [M, H, H], F32R, kind="ExternalInput").ap()
    psel_d = nc.dram_tensor("psel", [H, NPAIR, 128], F32R, kind="ExternalInput").ap()
    out_d = nc.dram_tensor("out", [BP, N, OD], F32, kind="ExternalOutput").ap()

    with tile.TileContext(nc) as tc:
        with (
            tc.tile_pool(name="wpool", bufs=1) as wp,
            tc.tile_pool(name="bpool", bufs=2) as bp,
            tc.tile_pool(name="cpool", bufs=2) as cp,
            tc.tile_pool(name="tp", bufs=2, space="PSUM") as tp,
            tc.tile_pool(name="pq", bufs=1, space="PSUM") as pq_pool,
            tc.tile_pool(name="ps", bufs=2, space="PSUM") as ps,
            tc.tile_pool(name="pd", bufs=1, space="PSUM") as pd_pool,
            tc.tile_pool(name="po", bufs=2, space="PSUM") as po,
        ):
            # --- one-time loads ---
            wq_a = wp.tile([128, 2, INNER], F32R)
            nc.sync.dma_start(wq_a, wqt_d[0:256].rearrange("(c p) i -> p c i", p=128))
            wq_b = wp.tile([64, INNER], F32R)
            nc.sync.dma_start(wq_b, wqt_d[256:QD])
            wk_sb = wp.tile([128, KC, INNER], F32R)
            nc.sync.dma_start(wk_sb, wkt_d.rearrange("(c p) i -> p c i", p=128))
            wv_sb = wp.tile([128, KC, INNER], F32R)
            nc.sync.dma_start(wv_sb, wvt_d.rearrange("(c p) i -> p c i", p=128))
            wo_sb = wp.tile([128, IC, OD], F32R)
            nc.sync.dma_start(wo_sb, wot_d.rearrange("(c p) o -> p c o", p=128))
            ident = wp.tile([128, 128], F32)
            nc.sync.dma_start(ident, ident_d)
            zoneh = wp.tile([M, H, H], F32R)
            nc.sync.dma_start(zoneh, zoneh_d)
            psel = wp.tile([H, NPAIR, 128], F32R)
            nc.sync.dma_start(psel, psel_d)
            bo_sb = wp.tile([128, OD], F32)
            nc.sync.dma_start(bo_sb, bo_d[None, :].to_broadcast([128, OD]))

            for b in range(BP):
                # --- per-batch K/V stage ---
                key_sb = bp.tile([M, KD], F32, tag="key_sb")
                nc.sync.dma_start(key_sb, key_d[b])
                val_sb = bp.tile([M, VD], F32, tag="val_sb")
                nc.sync.dma_start(val_sb, val_d[b])
                mb_sb = bp.tile([M, 1], F32, tag="mb_sb")
                nc.sync.dma_start(mb_sb, mb_d[b][:, None])

                keyT = bp.tile([128, KC, M], F32R, tag="keyT")
                valT = bp.tile([128, KC, M], F32R, tag="valT")
                for kc in range(KC):
                    pt = tp.tile([128, M], F32, tag="tp")
                    nc.tensor.transpose(
                        pt, key_sb[:, kc * 128 : (kc + 1) * 128], ident[:M, :M]
                    )
                    nc.any.tensor_copy(keyT[:, kc, :], pt)
                    pt2 = tp.tile([128, M], F32, tag="tp")
                    nc.tensor.transpose(
                        pt2, val_sb[:, kc * 128 : (kc + 1) * 128], ident[:M, :M]
                    )
                    nc.any.tensor_copy(valT[:, kc, :], pt2)

                pk = ps.tile([M, INNER], F32, tag="ps")
                for kc in range(KC):
                    nc.tensor.matmul(
                        pk,
                        keyT[:, kc, :],
                        wk_sb[:, kc, :],
                        start=(kc == 0),
                        stop=(kc == KC - 1),
                    )
                k_sb = bp.tile([M, INNER], F32, tag="k_sb")
                nc.any.tensor_copy(k_sb, pk)
                kT = bp.tile([128, IC, M], F32R, tag="kT")
                for ic in range(IC):
                    pt = tp.tile([128, M], F32, tag="tp")
                    nc.tensor.transpose(
                        pt, k_sb[:, ic * 128 : (ic + 1) * 128], ident[:M, :M]
                    )
                    nc.any.tensor_copy(kT[:, ic, :], pt)

                pv = ps.tile([M, INNER], F32, tag="ps")
                for kc in range(KC):
                    nc.tensor.matmul(
                        pv,
                        valT[:, kc, :],
                        wv_sb[:, kc, :],
                        start=(kc == 0),
                        stop=(kc == KC - 1),
                    )
                v_sb = bp.tile([M, INNER], F32R, tag="v_sb")
                nc.any.tensor_copy(v_sb, pv)

                # --- per-chunk pipeline ---
                for j in range(NCHUNKS):
                    nsl = slice(j * NCH, (j + 1) * NCH)
                    x_sb = cp.tile([128, NTT, QD], F32, tag="x_sb")
                    nc.sync.dma_start(
                        x_sb, x_d[b, nsl, :].rearrange("(t p) q -> p t q", p=128)
                    )
                    box_sb = cp.tile([128, NTT, M], F32, tag="box_sb")
                    nc.sync.dma_start(
                        box_sb, box_d[b, nsl, :].rearrange("(t p) m -> p t m", p=128)
                    )

                    xT0 = cp.tile([128, NCH], F32R, tag="xT0")
                    xT1 = cp.tile([128, NCH], F32R, tag="xT1")
                    xT2 = cp.tile([64, NCH], F32R, tag="xT2")
                    for t in range(NTT):
                        for lo, w, dst in ((0, 128, xT0), (128, 128, xT1), (256, 64, xT2)):
                            pt = tp.tile([w, 128], F32, tag="tp")
                            nc.tensor.transpose(pt, x_sb[:, t, lo : lo + w], ident)
                            nc.any.tensor_copy(dst[:, t * 128 : (t + 1) * 128], pt)

                    qT = cp.tile([128, IC, NCH], F32R, tag="qT")
                    for ic in range(IC):
                        pq = pq_pool.tile([128, NCH], F32, tag="pq")
                        isl = slice(ic * 128, (ic + 1) * 128)
                        nc.tensor.matmul(
                            pq, wq_a[:, 0, isl], xT0, start=True, stop=False
                        )
                        nc.tensor.matmul(
                            pq, wq_a[:, 1, isl], xT1, start=False, stop=False
                        )
                        nc.tensor.matmul(
                            pq, wq_b[:, isl], xT2, start=False, stop=True
                        )
                        nc.any.tensor_copy(qT[:, ic, :], pq)

                    boxT5 = cp.tile([M, NCH], F32, tag="boxT5")
                    for t in range(NTT):
                        pt = tp.tile([M, 128], F32, tag="tp")
                        nc.tensor.transpose(pt, box_sb[:, t, :], ident)
                        nc.any.tensor_copy(boxT5[:, t * 128 : (t + 1) * 128], pt)

                    e_all = cp.tile([M, H, NCH], F32R, tag="e_all")
                    pd = pd_pool.tile([H, NCH], F32, tag="pd")
                    for h in range(H):
                        pss = ps.tile([M, NCH], F32, tag="ps")
                        r0 = (h % 2) * 64
                        nc.tensor.matmul(
                            pss,
                            kT[r0 : r0 + 64, h // 2, :],
                            qT[r0 : r0 + 64, h // 2, :],
                            start=True,
                            stop=True,
                        )
                        nc.vector.scalar_tensor_tensor(
                            out=pss,
                            in0=pss,
                            scalar=mb_sb,
                            in1=boxT5,
                            op0=ALU.add,
                            op1=ALU.add,
                        )
                        nc.scalar.activation(e_all[:, h, :], pss, AF.Exp)
                        nc.tensor.matmul(
                            pd,
                            zoneh[:, h, :],
                            e_all[:, h, :],
                            start=(h == 0),
                            stop=(h == H - 1),
                        )

                    recip = cp.tile([H, NCH], F32R, tag="recip")
                    with nc.allow_low_precision(reason="fp32r softmax denom"):
                        nc.vector.reciprocal(recip, pd)

                    o_all = cp.tile([128, NPAIR, NCH], F32R, tag="o_all")
                    for p in range(NPAIR):
                        prb = po.tile([128, NCH], F32, tag="po")
                        nc.tensor.matmul(
                            prb, psel[:, p, :], recip, start=True, stop=True
                        )
                        rb_sb = cp.tile([128, NCH], F32, tag="rb_sb")
                        nc.scalar.copy(rb_sb, prb)
                        for r, h in ((0, 2 * p), (64, 2 * p + 1)):
                            pav = po.tile([128, NCH], F32, tag="po")
                            nc.tensor.matmul(
                                pav[0:64, :],
                                v_sb[:, h * C : (h + 1) * C],
                                e_all[:, h, :],
                                start=True,
                                stop=True,
                            )
                            nc.vector.tensor_tensor(
                                o_all[r : r + 64, p, :],
                                pav[0:64, :],
                                rb_sb[r : r + 64, :],
                                ALU.mult,
                            )

                    out_sb = cp.tile([128, NTT, OD], F32, tag="out_sb")
                    for t in range(NTT):
                        pf = pq_pool.tile([128, OD], F32, tag="pq")
                        for ic in range(IC):
                            nc.tensor.matmul(
                                pf,
                                o_all[:, ic, t * 128 : (t + 1) * 128],
                                wo_sb[:, ic, :],
                                start=(ic == 0),
                                stop=(ic == IC - 1),
                            )
                        nc.vector.tensor_add(out_sb[:, t, :], pf, bo_sb)
                    nc.sync.dma_start(
                        out_d[b, nsl, :].rearrange("(t p) o -> p t o", p=128), out_sb
                    )
    nc.compile()
    return nc


def host_inputs(x, key, value, mask, perl_box_masking_map, perl_road_masking_map,
                Wq, Wk, Wv, Wo, bo):
    """Host-side input marshaling: weight transposes, constant tables, mask
    bias. The road bias cancels inside the softmax and is dropped."""
    del perl_road_masking_map
    x = np.ascontiguousarray(np.asarray(x, np.float32))
    key = np.ascontiguousarray(np.asarray(key, np.float32))
    value = np.ascontiguousarray(np.asarray(value, np.float32))
    box = np.ascontiguousarray(np.asarray(perl_box_masking_map, np.float32) * np.float32(5.0))
    mask = np.asarray(mask, bool)
    maskbias = np.where(mask, np.float32(0.0), np.float32(MASK_NEG))
    maskbias = np.ascontiguousarray(maskbias.astype(np.float32))

    wqt = np.ascontiguousarray((np.asarray(Wq, np.float32) * np.float32(SCALE)).T)
    wkt = np.ascontiguousarray(np.asarray(Wk, np.float32).T)
    wvt = np.ascontiguousarray(np.asarray(Wv, np.float32).T)
    wot = np.ascontiguousarray(np.asarray(Wo, np.float32).T)
    bo = np.ascontiguousarray(np.asarray(bo, np.float32))

    ident = np.eye(128, dtype=np.float32)
    zoneh = np.zeros((M, H, H), np.float32)
    for h in range(H):
        zoneh[:, h, h] = 1.0
    psel = np.zeros((H, NPAIR, 128), np.float32)
    for p in range(NPAIR):
        psel[2 * p, p, 0:64] = 1.0
        psel[2 * p + 1, p, 64:128] = 1.0

    shared = {
        "WqT": wqt, "WkT": wkt, "WvT": wvt, "WoT": wot, "bo": bo,
        "ident": ident, "zoneh": zoneh, "psel": psel,
    }
    in_maps = []
    for c in range(NCORES):
        sl = slice(c * BP, (c + 1) * BP)
        m = {
            "x": x[sl], "key": key[sl], "value": value[sl],
            "maskbias": maskbias[sl], "box": box[sl],
        }
        m.update(shared)
        in_maps.append(m)
    return in_maps


_PROGRAM = None
LAST_RESULT = None


def kernel(**inputs):
    global _PROGRAM, LAST_RESULT
    from concourse.bass_utils import run_bass_kernel_spmd

    if _PROGRAM is None:
        _PROGRAM = build_program()
    in_maps = host_inputs(**inputs)
    trace = bool(int(os.environ.get("KERNEL_TRACE", "0")))
    res = run_bass_kernel_spmd(
        _PROGRAM, in_maps, list(range(NCORES)), trace=trace
    )
    LAST_RESULT = res
    out = np.concatenate([res.results[c]["out"] for c in range(NCORES)], axis=0)
    return np.ascontiguousarray(out.astype(np.float32))



# revision 17
# speedup vs baseline: 1.3187x; 1.3187x over previous
"""TRN2 Bass kernel: cross-attention (nn_CrossAttention_42047729828228).

Computes, per batch b:
  q = x @ Wq.T ; k = key @ Wk.T ; v = value @ Wv.T      (heads H=8, C=64)
  sim = einsum('nhc,mhc->hnm', q, k) * SCALE
  sim = where(mask, sim, -inf) + L1*box + L2*road
  out = einsum('hnm,mhc->nhc', softmax(sim, -1), v) @ Wo.T + bo

Device strategy: data-parallel over batch B=32 across 8 NeuronCores (4 each).

Key points vs a naive port:
 - road bias is constant along the softmax (key) axis -> cancels; dropped.
 - SCALE folded into Wq host-side; weights and activations in bf16 (matmuls
   run 1 cycle/row at any moving size; DVE gets 2x on 16-bit sbuf ops).
 - all operand transposes done host-side (xT, boxT, keyT, valT): the PE
   does zero transposes on device.
 - key mask: additive bias applied as the per-partition `bias` operand of
   the Exp activation (free on the Act engine).
 - box bias: accumulated into the scores PSUM by the PE itself via an
   identity-lhsT matmul (fp32r, full rate), fused into the sim accumulation
   group; no DVE/Act pass over the scores.
 - softmax denominators accumulated into one (8, n) psum via one-hot lhsT
   matmuls, split into two half-groups (heads 0-3 / 4-7) so reciprocals and
   the recip broadcast start mid-chunk; broadcast to 128 partitions with a
   pair-selector matmul; applied during the PSUM->SBUF move of AV.
 - single 7-bank PSUM ring pool (+1 dedicated denominator bank) so every
   producer/consumer pair is effectively double-buffered.
"""

import os
import sys

import numpy as np

sys.path.insert(0, "/opt/trn_rl_repo")

import concourse.bass as bass  # noqa: E402
import concourse.bacc as bacc  # noqa: E402
import concourse.mybir as mybir  # noqa: E402
import concourse.tile as tile  # noqa: E402

F32 = mybir.dt.float32
F32R = mybir.dt.float32r
BF16 = mybir.dt.bfloat16
AF = mybir.ActivationFunctionType
ALU = mybir.AluOpType

# Problem shapes (hardcoded; see module docstring).
B, N, M = 32, 1536, 80
QD, KD, VD = 320, 768, 768
H, C = 8, 64
INNER = H * C  # 512
OD = QD  # 320
SCALE = C**-0.5
NCORES = 8
BP = B // NCORES  # 4 batches per core
NCH = 512  # n-chunk (matmul moving dim)
NT = 128  # n-tile
NCHUNKS = N // NCH  # 3
NTT = NCH // NT  # 4
NPAIR = H // 2  # 4 head pairs
IC = INNER // 128  # 4 i-chunks
KC = KD // 128  # 6 kd-chunks
MASK_NEG = -1.0e9


def build_program(split_waits=True):  # split_waits kept for API compat
    nc = bacc.Bacc("TRN2", target_bir_lowering=False, debug=False, num_devices=NCORES)

    xT_d = nc.dram_tensor("xT", [BP, QD, N], BF16, kind="ExternalInput").ap()
    boxT_d = nc.dram_tensor("boxT", [BP, M, N], F32R, kind="ExternalInput").ap()
    keyT_d = nc.dram_tensor("keyT", [BP, KD, M], BF16, kind="ExternalInput").ap()
    valT_d = nc.dram_tensor("valT", [BP, VD, M], BF16, kind="ExternalInput").ap()
    mb_d = nc.dram_tensor("maskbias", [BP, M], F32, kind="ExternalInput").ap()
    wq01_d = nc.dram_tensor("Wq01", [128, 2, INNER], BF16, kind="ExternalInput").ap()
    wq2_d = nc.dram_tensor("Wq2", [64, INNER], BF16, kind="ExternalInput").ap()
    wk_d = nc.dram_tensor("Wk", [128, KC, INNER], BF16, kind="ExternalInput").ap()
    wv_d = nc.dram_tensor("Wv", [128, KC, INNER], BF16, kind="ExternalInput").ap()
    wo_d = nc.dram_tensor("Wo", [128, IC, OD], BF16, kind="ExternalInput").ap()
    bo_d = nc.dram_tensor("bo", [OD], F32, kind="ExternalInput").ap()
    identm_d = nc.dram_tensor("identm", [M, M], F32R, kind="ExternalInput").ap()
    zoneh_d = nc.dram_tensor("zoneh", [M, H, H], BF16, kind="ExternalInput").ap()
    psel_d = nc.dram_tensor("psel", [4, NPAIR, 128], BF16, kind="ExternalInput").ap()
    out_d = nc.dram_tensor("out", [BP, N, OD], F32, kind="ExternalOutput").ap()

    with tile.TileContext(nc) as tc:
        with (
            tc.tile_pool(name="wpool", bufs=1) as wp,
            tc.tile_pool(name="bpool", bufs=2) as bp,
            tc.tile_pool(name="cpool", bufs=2) as cp,
            tc.tile_pool(name="ring", bufs=7, space="PSUM") as pr,
            tc.tile_pool(name="pdp", bufs=1, space="PSUM") as pdp,
        ):
            # --- input-tile factories (so batch-0 DMAs can be hoisted
            # ahead of the cold-start weight loads) ---
            def load_batch(b):
                keyT = bp.tile([128, KC, M], BF16, tag="keyT", name="keyT")
                nc.sync.dma_start(keyT, keyT_d[b].rearrange("(c p) m -> p c m", p=128))
                valT = bp.tile([128, KC, M], BF16, tag="valT", name="valT")
                nc.sync.dma_start(valT, valT_d[b].rearrange("(c p) m -> p c m", p=128))
                mb_sb = bp.tile([M, 1], F32, tag="mb_sb", name="mb_sb")
                nc.sync.dma_start(mb_sb, mb_d[b][:, None])
                return keyT, valT, mb_sb

            def load_chunk(b, j):
                nsl = slice(j * NCH, (j + 1) * NCH)
                xt01 = cp.tile([128, 2, NCH], BF16, tag="xt01", name="xt01")
                nc.sync.dma_start(
                    xt01, xT_d[b, 0:256, nsl].rearrange("(c p) n -> p c n", p=128)
                )
                xt2 = cp.tile([64, NCH], BF16, tag="xt2", name="xt2")
                nc.sync.dma_start(xt2, xT_d[b, 256:QD, nsl])
                boxT = cp.tile([M, NCH], F32R, tag="boxT", name="boxT")
                nc.sync.dma_start(boxT, boxT_d[b, :, nsl])
                return xt01, xt2, boxT

            # --- loads, in first-use order ---
            wk_sb = wp.tile([128, KC, INNER], BF16)
            nc.scalar.dma_start(wk_sb, wk_d)
            batch0 = load_batch(0)
            wv_sb = wp.tile([128, KC, INNER], BF16)
            nc.scalar.dma_start(wv_sb, wv_d)
            chunk00 = load_chunk(0, 0)
            wq01 = wp.tile([128, 2, INNER], BF16)
            nc.scalar.dma_start(wq01, wq01_d)
            wq2 = wp.tile([64, INNER], BF16)
            nc.scalar.dma_start(wq2, wq2_d)
            identm = wp.tile([M, M], F32R)
            nc.scalar.dma_start(identm, identm_d)
            zoneh = wp.tile([M, H, H], BF16)
            nc.scalar.dma_start(zoneh, zoneh_d)
            psel = wp.tile([4, NPAIR, 128], BF16)
            nc.scalar.dma_start(psel, psel_d)
            wo_sb = wp.tile([128, IC, OD], BF16)
            nc.scalar.dma_start(wo_sb, wo_d)
            bo_sb = wp.tile([128, OD], F32)
            nc.scalar.dma_start(bo_sb, bo_d[None, :].to_broadcast([128, OD]))

            for b in range(BP):
                # --- per-batch K/V stage ---
                keyT, valT, mb_sb = batch0 if b == 0 else load_batch(b)

                # K projection directly in kT layout: [inner, m] chunks.
                pkT = pr.tile([128, IC, M], F32, tag="ring", name="pkT")
                for ic in range(IC):
                    isl = slice(ic * 128, (ic + 1) * 128)
                    for kc in range(KC):
                        nc.tensor.matmul(
                            pkT[:, ic, :],
                            wk_sb[:, kc, isl],
                            keyT[:, kc, :],
                            start=(kc == 0),
                            stop=(kc == KC - 1),
                        )
                kT = bp.tile([128, IC, M], BF16, tag="kT")
                nc.scalar.copy(kT, pkT)

                # V projection: v_sb [m, inner].
                pv = pr.tile([M, INNER], F32, tag="ring", name="pv")
                for kc in range(KC):
                    nc.tensor.matmul(
                        pv,
                        valT[:, kc, :],
                        wv_sb[:, kc, :],
                        start=(kc == 0),
                        stop=(kc == KC - 1),
                    )
                v_sb = bp.tile([M, INNER], BF16, tag="v_sb")
                nc.scalar.copy(v_sb, pv)

                # --- per-chunk pipeline ---
                for j in range(NCHUNKS):
                    nsl = slice(j * NCH, (j + 1) * NCH)
                    xt01, xt2, boxT = (
                        chunk00 if (b, j) == (0, 0) else load_chunk(b, j)
                    )

                    # Q projection -> qT [inner, n] chunks.
                    qT = cp.tile([128, IC, NCH], BF16, tag="qT")
                    for ic in range(IC):
                        isl = slice(ic * 128, (ic + 1) * 128)
                        pq = pr.tile([128, NCH], F32, tag="ring", name="pq")
                        nc.tensor.matmul(pq, wq01[:, 0, isl], xt01[:, 0, :],
                                         start=True, stop=False)
                        nc.tensor.matmul(pq, wq01[:, 1, isl], xt01[:, 1, :],
                                         start=False, stop=False)
                        nc.tensor.matmul(pq, wq2[:, isl], xt2,
                                         start=False, stop=True)
                        if ic % 2 == 0:
                            nc.vector.tensor_copy(qT[:, ic, :], pq)
                        else:
                            nc.scalar.copy(qT[:, ic, :], pq)

                    # Attention scores + softmax numerator/denominator.
                    e_all = cp.tile([M, H, NCH], BF16, tag="e_all")
                    pd = pdp.tile([68, NCH], F32, tag="pd", name="pd")
                    recip = cp.tile([4, 2, NCH], BF16, tag="recip")
                    o_all = cp.tile([128, IC, NCH], BF16, tag="o_all")
                    pav = [None] * NPAIR

                    def emit_head(h):
                        pss = pr.tile([M, NCH], F32, tag="ring", name=f"pss{h}")
                        nc.tensor.matmul(pss, identm, boxT, start=True, stop=False)
                        r0 = (h % 2) * 64
                        nc.tensor.matmul(
                            pss,
                            kT[r0 : r0 + 64, h // 2, :],
                            qT[r0 : r0 + 64, h // 2, :],
                            start=False,
                            stop=True,
                        )
                        nc.scalar.activation(e_all[:, h, :], pss, AF.Exp, bias=mb_sb)

                    def emit_zoneh(h):
                        # denominator half-groups: heads 0-3 -> pd[0:4],
                        # heads 4-7 -> pd[64:68] (psum out base must be 0/32/64).
                        zsl = slice(0, 4) if h < 4 else slice(4, 8)
                        osl = slice(0, 4) if h < 4 else slice(64, 68)
                        nc.tensor.matmul(
                            pd[osl, :],
                            zoneh[:, h, zsl],
                            e_all[:, h, :],
                            start=(h % 4 == 0),
                            stop=(h % 4 == 3),
                        )

                    def emit_recip(half):
                        psl = slice(0, 4) if half == 0 else slice(64, 68)
                        with nc.allow_low_precision(reason="bf16 softmax denom"):
                            nc.vector.reciprocal(recip[:, half, :], pd[psl, :])

                    rb_sb = cp.tile([128, NPAIR, NCH], BF16, tag="rb_sb")

                    def emit_rb(p):
                        rb = pr.tile([128, NCH], F32, tag="ring", name=f"rb{p}")
                        nc.tensor.matmul(rb, psel[:, p, :], recip[:, p // 2, :],
                                         start=True, stop=True)
                        nc.scalar.copy(rb_sb[:, p, :], rb)

                    def emit_av_pair(p):
                        pav2 = pr.tile([128, NCH], F32, tag="ring", name=f"pav{p}")
                        for r, h in ((0, 2 * p), (64, 2 * p + 1)):
                            nc.tensor.matmul(
                                pav2[r : r + 64, :],
                                v_sb[:, h * C : (h + 1) * C],
                                e_all[:, h, :],
                                start=True,
                                stop=True,
                            )
                        pav[p] = pav2

                    def emit_norm_pair(p):
                        nc.vector.tensor_tensor(
                            o_all[:, p, :], pav[p], rb_sb[:, p, :], ALU.mult
                        )

                    # software-pipelined emission: zoneh deferred 2 heads
                    # behind its exp so the PE never waits on the Act engine.
                    emit_head(0)
                    emit_head(1)
                    for h in range(2, 8):
                        emit_head(h)
                        emit_zoneh(h - 2)
                        if h == 5:
                            emit_recip(0)
                        if h == 6:
                            emit_rb(0)
                            emit_av_pair(0)
                            emit_norm_pair(0)
                        if h == 7:
                            emit_rb(1)
                            emit_av_pair(1)
                            emit_norm_pair(1)
                    emit_zoneh(6)
                    emit_zoneh(7)
                    emit_recip(1)
                    emit_av_pair(2)
                    emit_av_pair(3)
                    emit_rb(2)
                    emit_rb(3)
                    emit_norm_pair(2)
                    emit_norm_pair(3)

                    # Output projection in two half-accumulations per n-tile
                    # (ic 0,1 need only head pairs 0,1; ic 2,3 the rest),
                    # bias folded into the PSUM->SBUF move on DVE.
                    out_sb = cp.tile([128, NTT, OD], F32, tag="out_sb")
                    pf = [None] * NTT
                    for t in range(NTT):
                        tsl = slice(t * 128, (t + 1) * 128)
                        pf[t] = pr.tile([128, OD], F32, tag="ring", name=f"pf{t}")
                        for ic in (0, 1):
                            nc.tensor.matmul(
                                pf[t],
                                o_all[:, ic, tsl],
                                wo_sb[:, ic, :],
                                start=(ic == 0),
                                stop=False,
                            )
                    for t in range(NTT):
                        tsl = slice(t * 128, (t + 1) * 128)
                        for ic in (2, 3):
                            nc.tensor.matmul(
                                pf[t],
                                o_all[:, ic, tsl],
                                wo_sb[:, ic, :],
                                start=False,
                                stop=(ic == IC - 1),
                            )
                        nc.vector.tensor_tensor(out_sb[:, t, :], pf[t], bo_sb, ALU.add)
                        if (b, j) == (BP - 1, NCHUNKS - 1):
                            tn = slice(j * NCH + t * 128, j * NCH + (t + 1) * 128)
                            nc.sync.dma_start(out_d[b, tn, :], out_sb[:, t, :])
                    if (b, j) != (BP - 1, NCHUNKS - 1):
                        nc.sync.dma_start(
                            out_d[b, nsl, :].rearrange("(t p) o -> p t o", p=128),
                            out_sb,
                        )
    nc.compile()
    return nc


def host_inputs(x, key, value, mask, perl_box_masking_map, perl_road_masking_map,
                Wq, Wk, Wv, Wo, bo):
    """Host-side marshaling: transposes, bf16 casts, constant tables, mask
    bias. The road bias cancels inside the softmax and is dropped."""
    import ml_dtypes

    del perl_road_masking_map
    bf16 = ml_dtypes.bfloat16

    x = np.asarray(x, np.float32)
    xT = np.ascontiguousarray(x.transpose(0, 2, 1)).astype(bf16)  # [B, QD, N]
    key = np.asarray(key, np.float32)
    keyT = np.ascontiguousarray(key.transpose(0, 2, 1)).astype(bf16)  # [B, KD, M]
    value = np.asarray(value, np.float32)
    valT = np.ascontiguousarray(value.transpose(0, 2, 1)).astype(bf16)
    box = np.asarray(perl_box_masking_map, np.float32) * np.float32(5.0)
    boxT = np.ascontiguousarray(box.transpose(0, 2, 1))  # [B, M, N] f32
    mask = np.asarray(mask, bool)
    maskbias = np.where(mask, np.float32(0.0), np.float32(MASK_NEG))
    maskbias = np.ascontiguousarray(maskbias.astype(np.float32))

    wqt = (np.asarray(Wq, np.float32) * np.float32(SCALE)).T  # [QD, INNER]
    wq01 = np.ascontiguousarray(
        wqt[0:256].reshape(2, 128, INNER).transpose(1, 0, 2)).astype(bf16)
    wq2 = np.ascontiguousarray(wqt[256:QD]).astype(bf16)
    wkt = np.asarray(Wk, np.float32).T  # [KD, INNER]
    wk = np.ascontiguousarray(
        wkt.reshape(KC, 128, INNER).transpose(1, 0, 2)).astype(bf16)
    wvt = np.asarray(Wv, np.float32).T
    wv = np.ascontiguousarray(
        wvt.reshape(KC, 128, INNER).transpose(1, 0, 2)).astype(bf16)
    wot = np.asarray(Wo, np.float32).T  # [INNER, OD]
    wo = np.ascontiguousarray(
        wot.reshape(IC, 128, OD).transpose(1, 0, 2)).astype(bf16)
    bo = np.ascontiguousarray(np.asarray(bo, np.float32))

    identm = np.eye(M, dtype=np.float32)
    zoneh = np.zeros((M, H, H), np.float32)
    for h in range(H):
        zoneh[:, h, h] = 1.0
    zoneh = zoneh.astype(bf16)
    # pair p reads recip rows: p0 -> 0,1 / p1 -> 2,3 (half 0);
    # p2 -> 0,1 / p3 -> 2,3 (half 1). K=2 contraction for p2.
    # pair p contracts K=4 over its recip half: rows 0,1 for even pairs,
    # rows 2,3 for odd pairs.
    psel = np.zeros((4, NPAIR, 128), np.float32)
    for p in range(NPAIR):
        psel[2 * (p % 2), p, 0:64] = 1.0
        psel[2 * (p % 2) + 1, p, 64:128] = 1.0
    psel = psel.astype(bf16)

    shared = {
        "Wq01": wq01, "Wq2": wq2, "Wk": wk, "Wv": wv, "Wo": wo, "bo": bo,
        "identm": identm, "zoneh": zoneh, "psel": psel,
    }
    in_maps = []
    for c in range(NCORES):
        sl = slice(c * BP, (c + 1) * BP)
        m = {
            "xT": xT[sl], "keyT": keyT[sl], "valT": valT[sl],
            "maskbias": maskbias[sl], "boxT": boxT[sl],
        }
        m.update(shared)
        in_maps.append(m)
    return in_maps


_PROGRAM = None
LAST_RESULT = None


def kernel(**inputs):
    global _PROGRAM, LAST_RESULT
    from concourse.bass_utils import run_bass_kernel_spmd

    if _PROGRAM is None:
        _PROGRAM = build_program()
    in_maps = host_inputs(**inputs)
    trace = bool(int(os.environ.get("KERNEL_TRACE", "0")))
    res = run_bass_kernel_spmd(
        _PROGRAM, in_maps, list(range(NCORES)), trace=trace
    )
    LAST_RESULT = res
    out = np.concatenate([res.results[c]["out"] for c in range(NCORES)], axis=0)
    return np.ascontiguousarray(out.astype(np.float32))
